# revision 15
# baseline (speedup 1.0000x reference)
"""Trainium2 Bass kernel for nn_ExNASWrapper_59700045414555 (topk_masking).

Self-contained: takes full inputs, shards across 8 NeuronCores, runs one SPMD
Bass program (3 small collectives: AllReduce sig, AllToAll x_pool, ReduceScatter
logits), returns the full [128, 1000] output.

Key algebraic facts used (validated numerically against the reference):
  * scatter of the selected-channel conv2 == full conv2 masked by channel;
    relu commutes, so computing ONLY the 48 selected channels is exact.
  * K_FC (12451) exceeds the number of nonzero features after channel
    masking (48*256 = 12288), and zero-score features are exactly-zero
    columns of x_flat, so the feature top-k is an exact no-op:
    out = x_flat @ fc_w.T + fc_b over the 48 kept channel slabs only.
  * softplus is monotonic, so channel selection can use pre-softplus scores.
"""
import os
import sys

for _p in ("/opt/trn_rl_repo",):
    if _p not in sys.path:
        sys.path.insert(0, _p)

import numpy as np

import concourse.bacc as bacc
import concourse.bass as bass
import concourse.mybir as mybir
import concourse.tile as tile
from concourse import bass_utils
from concourse.bass import IndirectOffsetOnAxis

F32 = mybir.dt.float32
F32R = mybir.dt.float32r      # full-rate fp32 streaming mode (N>=256)
I32 = mybir.dt.int32
AF = mybir.ActivationFunctionType
OP = mybir.AluOpType

B, CIN, H, W = 128, 3, 32, 32
C1, C2 = 64, 256
NCLS = 1000
NCORES = 8
BL = B // NCORES                 # 16 images per core
HP = WP = 34                     # zero-padded spatial
IMG = HP * WP                    # 1156
NPIX = BL * IMG                  # 18496
KC = 48                          # kept conv2 channels
SPW = 256 // NCORES              # 32 pooled-spatial positions per core
NT = 2 * BL                      # 32 matmul tiles per core (512 px each)
NQ = 4                           # conv1 input quarters (double-buffered)
NFC = 12                         # FC K-chunks of 128 features
FC_BUFS = 12                     # prefetched fc-weight tiles

# conv2 taps; chunks pair taps whose padded-flat deltas differ by +1 so that
# rows 64-127 of x1sb (a +1-shifted copy) supply the second tap of the pair.
TAPS = [(dy, dx) for dy in range(3) for dx in range(3)]
PAIR_CHUNKS = [(0, 1), (3, 4), (6, 7)]      # tap-index pairs (delta, delta+1)
SINGLE_CHUNKS = [2, 5, 8]

REPL = [list(range(NCORES))]


def _host_consts():
    Lstrict = (np.arange(256)[:, None] < np.arange(256)[None, :]).astype(np.float32)
    kiota = np.broadcast_to(np.arange(KC, dtype=np.float32), (128, KC)).copy()
    chiota = np.arange(256, dtype=np.float32).reshape(2, 128, 1)
    expand = np.zeros((KC, NFC * 128), np.float32)
    for t in range(NFC):
        for p in range(128):
            expand[4 * t + p // 32, t * 128 + p] = 1.0
    spmod = (np.arange(128, dtype=np.float32) % SPW).reshape(128, 1)
    ones_row = np.ones((1, 128), np.float32)
    return Lstrict, kiota, chiota, expand, spmod, ones_row


def _host_prep(inputs):
    x0 = np.ascontiguousarray(inputs['x0'], np.float32)
    w1 = np.ascontiguousarray(inputs['w1'], np.float32)
    b1 = np.ascontiguousarray(inputs['b1'], np.float32)
    gate_w = np.ascontiguousarray(inputs['gate_w'], np.float32)
    w2 = np.ascontiguousarray(inputs['w2'], np.float32)
    b2 = np.ascontiguousarray(inputs['b2'], np.float32)
    fc_w = np.ascontiguousarray(inputs['fc_w'], np.float32)
    fc_b = np.ascontiguousarray(inputs['fc_b'], np.float32)

    # conv1 im2col per core: [27, BL*1024], row r = c*9 + ky*3 + kx
    x0p = np.zeros((B, CIN, HP, WP), np.float32)
    x0p[:, :, 1:33, 1:33] = x0
    cols = np.empty((NCORES, 27, BL * 1024), np.float32)
    scratch = np.empty((CIN, 3, 3, BL, 32, 32), np.float32)
    for j in range(NCORES):
        xs = x0p[j * BL:(j + 1) * BL]
        for ky in range(3):
            for kx in range(3):
                scratch[:, ky, kx] = xs[:, :, ky:ky + 32, kx:kx + 32].transpose(1, 0, 2, 3)
        cols[j] = scratch.reshape(27, BL * 1024)

    w1T = np.ascontiguousarray(w1.reshape(C1, 27).T)            # [27, 64]
    gate_wT = np.ascontiguousarray(gate_w.T)                    # [64, 256]
    w2t = np.ascontiguousarray(w2.transpose(2, 3, 0, 1)).reshape(9, 2, 128, C1)
    # fc shard per core: [256ch, SPW sp, 1000] -> [8192, 1000]
    fcv = fc_w.reshape(NCLS, C2, 256).transpose(1, 2, 0)        # view [256,256,1000]
    fcs = [np.ascontiguousarray(fcv[:, j * SPW:(j + 1) * SPW, :]).reshape(C2 * SPW, NCLS)
           for j in range(NCORES)]
    fcb8 = (fc_b / NCORES).reshape(1, NCLS).astype(np.float32)

    Lstrict, kiota, chiota, expand, spmod, ones_row = _host_consts()
    common = dict(w1T=w1T, b1=b1.reshape(C1, 1), gate_wT=gate_wT, w2t=w2t,
                  b2=b2.reshape(2, 128, 1), fcb8=fcb8,
                  Lstrict=Lstrict.reshape(2, 128, 256), kiota=kiota,
                  chiota=chiota, expand=expand, spmod=spmod, ones_row=ones_row,
                  zeros=np.zeros((C1, 2 * 543), np.float32))
    in_maps = []
    for j in range(NCORES):
        m = dict(common)
        m['x0i'] = np.ascontiguousarray(
            cols[j].reshape(27, NQ, BL * 1024 // NQ).transpose(1, 0, 2))
        m['fcs'] = fcs[j]
        in_maps.append(m)
    return in_maps


def _build_nc(reps=1):
    nc = bacc.Bacc("TRN2", target_bir_lowering=False, debug=False,
                   enable_asserts=False, num_devices=NCORES)
    aps = {}

    def din(name, shape, dt=F32):
        aps[name] = nc.dram_tensor(name, list(shape), dt, kind="ExternalInput").ap()

    din('x0i', (NQ, 27, BL * 1024 // NQ), F32R)
    din('w1T', (27, C1), F32R)
    din('b1', (C1, 1))
    din('gate_wT', (C1, 256))
    din('w2t', (9, 2, 128, C1))
    din('b2', (2, 128, 1))
    din('fcs', (C2 * SPW, NCLS), F32R)
    din('fcb8', (1, NCLS))
    din('Lstrict', (2, 128, 256))
    din('kiota', (128, KC))
    din('chiota', (2, 128, 1))
    din('expand', (KC, NFC * 128))
    din('spmod', (128, 1))
    din('ones_row', (1, 128))
    din('zeros', (C1, 2 * 543), F32R)
    out_ap = nc.dram_tensor('out', [BL, NCLS], F32, kind="ExternalOutput").ap()

    with tile.TileContext(nc) as tc:
        for _ in range(reps):
            _body(tc, aps, out_ap)
    nc.compile()
    return nc


def _body(tc, aps, out_ap):
    nc = tc.nc
    with (
        tc.tile_pool(name="const", bufs=1) as cpool,
        tc.tile_pool(name="x0", bufs=2) as x0pool,
        tc.tile_pool(name="x1", bufs=1) as x1pool,
        tc.tile_pool(name="work", bufs=2) as work,
        tc.tile_pool(name="fcw", bufs=FC_BUFS) as fcw_pool,
        tc.tile_pool(name="ps_conv", bufs=3, space="PSUM") as ps_conv,
        tc.tile_pool(name="ps_fc", bufs=1, space="PSUM") as ps_fc,
        tc.tile_pool(name="ps_sm", bufs=2, space="PSUM") as ps_sm,
        tc.tile_pool(name="dram", bufs=1, space="DRAM") as dram,
    ):
        # ---------------- constants & weights ----------------
        def load(name):
            src = aps[name]
            t = cpool.tile(list(src.shape), src.dtype, tag=name)
            nc.sync.dma_start(t[:], src)
            return t

        w1T = load('w1T')                       # [27, 64]
        b1 = load('b1')                         # [64, 1]
        gate_wT = load('gate_wT')               # [64, 256]
        w2t = cpool.tile([128, 9, 2, C1], F32, tag='w2t')
        nc.sync.dma_start(w2t[:], aps['w2t'].rearrange("t h p o -> p t h o"))
        b2 = cpool.tile([128, 2], F32, tag='b2')
        nc.sync.dma_start(b2[:], aps['b2'].rearrange("h p one -> p (h one)"))
        fcb8 = load('fcb8')                     # [1, 1000]
        Lk = cpool.tile([128, 2, 256], F32, tag='L')
        nc.sync.dma_start(Lk[:], aps['Lstrict'].rearrange("h p m -> p h m"))
        kiota = load('kiota')                   # [128, 48]
        chiota = cpool.tile([128, 2], F32, tag='chiota')
        nc.sync.dma_start(chiota[:], aps['chiota'].rearrange("h p one -> p (h one)"))
        expand = load('expand')                 # [48, 1536]
        spmod = load('spmod')                   # [128, 1]
        ones_row = load('ones_row')             # [1, 128]

        # ---------------- conv1 + sig accumulation ----------------
        # x1sb rows 0-63: padded relu(conv1(x0)); rows 64-127: shifted by +1
        x1sb = x1pool.tile([128, NPIX], F32R)
        x1v = x1sb[:].rearrange("p (i y x) -> p i y x", i=BL, y=HP)
        # zero the pad ring via DMA from a zeros input (memset can't write
        # float32r, and fp32r consumers require fp32r-dtype producers).
        # Rows y=0/y=33 as clean 3-dim DMAs; the x=0/x=33 columns as one
        # stride-34 DMA over the contiguous (x=33,y)+(x=0,y+1) pairs.
        zsrc = aps['zeros']
        zrow = zsrc[:, 0:BL * WP].rearrange("p (i x) -> p i x", i=BL)
        nc.sync.dma_start(x1v[0:C1, :, 0, :], zrow)
        nc.sync.dma_start(x1v[0:C1, :, 33, :], zrow)
        _x1 = x1sb[:]
        NPR = 543
        nc.sync.dma_start(
            bass.AP(_x1.tensor, _x1.offset + 33, [[NPIX, C1], [WP, NPR], [1, 2]]),
            zsrc[:, 0:2 * NPR].rearrange("p (k two) -> p k two", two=2))

        sigacc = work.tile([C1, NT], F32, tag="sigacc")
        zero1 = work.tile([C1, 1], F32, tag="zero1")
        nc.vector.memset(zero1[:], 0.0)
        TQ = NT // NQ                            # matmul tiles per quarter
        for q in range(NQ):
            x0q = x0pool.tile([27, BL * 1024 // NQ], F32R, tag="x0q")
            nc.sync.dma_start(x0q[:], aps['x0i'][q])
            for k in range(TQ):
                n = q * TQ + k
                i, h = n // 2, n % 2
                ps = ps_conv.tile([C1, 512], F32, tag="conv")
                nc.tensor.matmul(ps[:], lhsT=w1T[:],
                                 rhs=x0q[:, k * 512:(k + 1) * 512],
                                 start=True, stop=True)
                dst = x1v[0:C1, i, 16 * h + 1:16 * h + 17, 1:33]
                psv = ps[:].rearrange("c (y x) -> c y x", y=16)
                if n % 2 == 0:
                    nc.scalar.activation(dst, psv, AF.Relu, bias=b1[:],
                                         accum_out=sigacc[:, n:n + 1])
                else:
                    # relu(ps + b1) on DVE: (ps add b1) max 0broadcast
                    nc.vector.scalar_tensor_tensor(
                        out=dst, in0=psv, scalar=b1[:],
                        in1=zero1[:, 0:1].to_broadcast([C1, 16, 32]),
                        op0=OP.add, op1=OP.max,
                        accum_out=sigacc[:, n:n + 1])
        # +1-shifted dup; the one missing tail element is image-15 pad (zeroed)
        nc.sync.dma_start(x1sb[64:128, NPIX - 1:NPIX], zsrc[:, 0:1])
        nc.sync.dma_start(x1sb[64:128, 0:NPIX - 1], x1sb[0:C1, 1:NPIX])

        sig_p = work.tile([C1, 1], F32, tag="sig")
        nc.vector.tensor_reduce(sig_p[:], sigacc[:], axis=mybir.AxisListType.X,
                                op=OP.add)
        nc.vector.tensor_scalar_mul(sig_p[:], sig_p[:], 1.0 / (B * H * W))

        # ---------------- AllReduce sig ----------------
        sig_in = dram.tile([C1, 1], F32)
        sig_out = dram.tile([C1, 1], F32)
        nc.sync.dma_start(sig_in[:], sig_p[:])
        nc.gpsimd.collective_compute("AllReduce", OP.add, replica_groups=REPL,
                                     ins=[sig_in[:]], outs=[sig_out[:]])
        sig = work.tile([C1, 1], F32, tag="sig")
        nc.sync.dma_start(sig[:], sig_out[:])

        # ---------------- scores, threshold, mask, S ----------------
        ps_sf = ps_sm.tile([1, 256], F32, tag="sm")
        nc.tensor.matmul(ps_sf[:], lhsT=sig[:], rhs=gate_wT[:], start=True, stop=True)
        scr = work.tile([1, 256], F32, tag="scr")
        nc.vector.tensor_copy(scr[:], ps_sf[:])
        maxes = work.tile([1, 48], F32, tag="maxes")
        for r in range(6):
            nc.vector.max(out=maxes[0:1, 8 * r:8 * r + 8], in_=scr[:])
            if r < 5:
                nc.vector.match_replace(out=scr[:],
                                        in_to_replace=maxes[0:1, 8 * r:8 * r + 8],
                                        in_values=scr[:], imm_value=-1e30)
        scores = work.tile([128, 2], F32, tag="scores")
        for h in range(2):
            ps_s = ps_sm.tile([128, 1], F32, tag="sm")
            nc.tensor.matmul(ps_s[:], lhsT=gate_wT[:, 128 * h:128 * (h + 1)],
                             rhs=sig[:], start=True, stop=True)
            nc.vector.tensor_copy(scores[:, h:h + 1], ps_s[:])
        ps_thr = ps_sm.tile([128, 1], F32, tag="sm")
        nc.tensor.matmul(ps_thr[:], lhsT=ones_row[:], rhs=maxes[0:1, 47:48],
                         start=True, stop=True)
        thrb = work.tile([128, 1], F32, tag="thrb")
        nc.vector.tensor_copy(thrb[:], ps_thr[:])
        mask = work.tile([128, 2], F32, tag="mask")
        nc.vector.tensor_tensor(out=mask[:], in0=scores[:],
                                in1=thrb[:].to_broadcast([128, 2]), op=OP.is_ge)
        # rank[m] = #selected channels with index < m; S = one-hot(rank)*mask
        S = work.tile([128, 2, KC], F32, tag="S")
        for h in range(2):
            ps_r = ps_sm.tile([128, 1], F32, tag="sm")
            for kh in range(2):
                nc.tensor.matmul(ps_r[:], lhsT=Lk[:, kh, 128 * h:128 * (h + 1)],
                                 rhs=mask[:, kh:kh + 1],
                                 start=(kh == 0), stop=(kh == 1))
            nc.vector.scalar_tensor_tensor(out=S[:, h, :], in0=kiota[:],
                                           scalar=ps_r[:],
                                           in1=mask[:, h:h + 1].to_broadcast([128, KC]),
                                           op0=OP.is_equal, op1=OP.mult)
        ps_k = ps_sm.tile([KC, 1], F32, tag="sm")
        for h in range(2):
            nc.tensor.matmul(ps_k[:], lhsT=S[:, h, :], rhs=chiota[:, h:h + 1],
                             start=(h == 0), stop=(h == 1))
        kept = work.tile([KC, 1], F32, tag="kept")
        nc.vector.tensor_copy(kept[:], ps_k[:])
        ps_b = ps_sm.tile([KC, 1], F32, tag="sm")
        for h in range(2):
            nc.tensor.matmul(ps_b[:], lhsT=S[:, h, :], rhs=b2[:, h:h + 1],
                             start=(h == 0), stop=(h == 1))
        b2sel = work.tile([KC, 1], F32, tag="b2sel")
        nc.vector.tensor_copy(b2sel[:], ps_b[:])

        # selected conv2 weights: W_sel[t] = w2t[t]^T S = [64, 48]; pair
        # chunks land as [128, 48] (tap d on rows 0-63, tap d+1 on 64-127)
        wsel_pairs = []
        for (ta, tb) in PAIR_CHUNKS:
            ps_w = ps_sm.tile([128, KC], F32, tag="sm")
            for (t, off) in ((ta, 0), (tb, 64)):
                for h in range(2):
                    nc.tensor.matmul(ps_w[off:off + C1, :], lhsT=w2t[:, t, h, :],
                                     rhs=S[:, h, :], start=(h == 0), stop=(h == 1),
                                     tile_position=(0, off))
            wt = work.tile([128, KC], F32R, tag=f"wp{ta}")
            nc.vector.tensor_copy(wt[:], ps_w[:])
            wsel_pairs.append(wt)
        wsel_single = []
        for t in SINGLE_CHUNKS:
            ps_w = ps_sm.tile([C1, KC], F32, tag="sm")
            for h in range(2):
                nc.tensor.matmul(ps_w[:], lhsT=w2t[:, t, h, :], rhs=S[:, h, :],
                                 start=(h == 0), stop=(h == 1))
            wt = work.tile([C1, KC], F32R, tag=f"ws{t}")
            nc.vector.tensor_copy(wt[:], ps_w[:])
            wsel_single.append(wt)

        # ---------------- fc weight gather (overlaps conv2) ----------------
        fcrhs = []
        for t in range(NFC):
            ps_i = ps_sm.tile([128, 1], F32, tag="sm")
            nc.tensor.matmul(ps_i[:], lhsT=expand[:, t * 128:(t + 1) * 128],
                             rhs=kept[:], start=True, stop=True)
            idxf = work.tile([128, 1], F32, tag="idxf")
            nc.vector.scalar_tensor_tensor(out=idxf[:], in0=ps_i[:],
                                           scalar=float(SPW), in1=spmod[:],
                                           op0=OP.mult, op1=OP.add)
            idxi = work.tile([128, 1], I32, tag="idxi")
            nc.vector.tensor_copy(idxi[:], idxf[:])
            rt = fcw_pool.tile([128, NCLS], F32R, tag="fcrhs")
            nc.gpsimd.indirect_dma_start(
                out=rt[:], out_offset=None, in_=aps['fcs'],
                in_offset=IndirectOffsetOnAxis(ap=idxi[:, 0:1], axis=0))
            fcrhs.append(rt)

        # ---------------- conv2 + maxpool + transposed eviction ----------------
        xpoolT = x1pool.tile([KC, 256, BL], F32R, tag="xpoolT")
        x1f = x1sb[:].rearrange("p (i y x) -> p i y x", i=BL, y=HP)
        for n in range(NT):
            i, h = n // 2, n % 2
            ps = ps_conv.tile([KC, 512], F32, tag="conv")
            nmm = 0
            for ci, (ta, _tb) in enumerate(PAIR_CHUNKS):
                dy, dx = TAPS[ta]
                rhs = x1f[0:128, i, 16 * h + dy:16 * h + dy + 16, dx:dx + 32]
                nc.tensor.matmul(ps[:], lhsT=wsel_pairs[ci][:], rhs=rhs,
                                 start=(nmm == 0), stop=False)
                nmm += 1
            for ci, t in enumerate(SINGLE_CHUNKS):
                dy, dx = TAPS[t]
                rhs = x1f[0:C1, i, 16 * h + dy:16 * h + dy + 16, dx:dx + 32]
                nc.tensor.matmul(ps[:], lhsT=wsel_single[ci][:], rhs=rhs,
                                 start=False, stop=(ci == 2))
            # maxpool 2x2: [48, 16y, 32x] -> [48, 8, 16], then relu(. + b2sel)
            # (two tensor_reduce ops: DVE may read only ONE psum input)
            pv = ps[:].rearrange("c (y xx xs) -> c y xx xs", y=16, xs=2)
            t1 = work.tile([KC, 16, 16], F32, tag="ptmp")
            nc.vector.tensor_reduce(t1[:], pv, axis=mybir.AxisListType.X,
                                    op=OP.max)
            t2 = work.tile([KC, 8, 16], F32, tag="ptmp2")
            nc.vector.tensor_reduce(
                t2[:], t1[:].rearrange("c (yy ys) x -> c yy x ys", ys=2),
                axis=mybir.AxisListType.X, op=OP.max)
            # xpoolT[c, (8h+yp)*16+xp, i] = relu(t2 + b2sel)
            xpv = xpoolT[:].rearrange("c (yp xp) b -> c yp xp b", yp=16)
            nc.scalar.activation(xpv[:, 8 * h:8 * h + 8, :, i], t2[:],
                                 AF.Relu, bias=b2sel[:])

        # ---------------- AllToAll (reshard: batch -> pooled-spatial) --------
        a2a_in = dram.tile([NCORES, KC, SPW, BL], F32R)
        a2a_out = dram.tile([NCORES, KC, SPW, BL], F32R)
        nc.sync.dma_start(a2a_in[:].rearrange("j c s b -> c j s b"),
                          xpoolT[:].rearrange("c (j s) b -> c j s b", j=NCORES))
        nc.gpsimd.collective_compute("AllToAll", OP.bypass, replica_groups=REPL,
                                     ins=[a2a_in[:]], outs=[a2a_out[:]])

        # ---------------- FC partials [128b, 1000] ----------------
        po0 = ps_fc.tile([128, 500], F32, tag="po0")
        po1 = ps_fc.tile([128, 500], F32, tag="po1")
        for t in range(NFC):
            xt = work.tile([128, 8, BL], F32R, tag="xt")
            src = a2a_out[:, 4 * t:4 * t + 4, :, :].rearrange("i c s b -> (c s) i b")
            nc.sync.dma_start(xt[:], src)
            xtf = xt[:].rearrange("p i b -> p (i b)")
            nc.tensor.matmul(po0[:], lhsT=xtf, rhs=fcrhs[t][:, 0:500],
                             start=(t == 0), stop=False)
            nc.tensor.matmul(po1[:], lhsT=xtf, rhs=fcrhs[t][:, 500:1000],
                             start=(t == 0), stop=False)
        # += fc_b / 8 so the ReduceScatter sum adds exactly one fc_b
        nc.tensor.matmul(po0[:], lhsT=ones_row[:], rhs=fcb8[0:1, 0:500],
                         start=False, stop=True)
        nc.tensor.matmul(po1[:], lhsT=ones_row[:], rhs=fcb8[0:1, 500:1000],
                         start=False, stop=True)

        outsb = work.tile([128, NCLS], F32, tag="outsb")
        nc.vector.tensor_copy(outsb[:, 0:500], po0[:])
        nc.vector.tensor_copy(outsb[:, 500:1000], po1[:])

        # ---------------- ReduceScatter over batch ----------------
        rs_in = dram.tile([B, NCLS], F32)
        rs_out = dram.tile([BL, NCLS], F32)
        nc.sync.dma_start(rs_in[:], outsb[:])
        nc.gpsimd.collective_compute("ReduceScatter", OP.add, replica_groups=REPL,
                                     ins=[rs_in[:]], outs=[rs_out[:]])
        nc.sync.dma_start(out_ap, rs_out[:])


_NC_CACHE = {}


def _get_nc(reps=1):
    if reps not in _NC_CACHE:
        _NC_CACHE[reps] = _build_nc(reps)
    return _NC_CACHE[reps]


def kernel(**inputs) -> np.ndarray:
    reps = int(os.environ.get("KERNEL_REPS", "1"))
    in_maps = _host_prep(inputs)
    nc = _get_nc(reps)
    res = bass_utils.run_bass_kernel_spmd(
        nc, in_maps, core_ids=list(range(NCORES)),
        trace=bool(int(os.environ.get("KERNEL_TRACE", "0"))))
    out = np.concatenate([res.results[j]['out'] for j in range(NCORES)], axis=0)
    if res.exec_time_ns is not None:
        print(f"HW exec time: {res.exec_time_ns} ns")
    return out


if __name__ == '__main__':
    d = np.load('/root/problem/ref_inputs.npz')
    inputs = {k: d[k] for k in d.files}
    expected = np.load('/root/problem/ref_out.npy')
    actual = kernel(**inputs)
    rel = np.linalg.norm(actual - expected) / np.linalg.norm(expected)
    print('Relative error:', rel)
